# revision 20
# baseline (speedup 1.0000x reference)
"""HAN forward on 8 trn2 NeuronCores — host-staged edge-stream version.

Dst nodes sharded across 8 cores (6250 each), assigned to 49 blocks of 128
by a load balancer so per-(block, metapath) edge counts are uniform. The
host materializes, per core and metapath, the edge message stream
et[slot] = h[src] * 1/sqrt(deg_out*deg_in) in bf16, laid out
partition-major ([128 lanes, chunk*256 cols]) so the device streams it
with large fully-contiguous HWDGE DMAs — no on-device gather (the SWDGE
descriptor generation on GpSimd was the previous bottleneck). Selection
matrices are binary (norm folded into the stream): one batched DVE
tensor_tensor is_equal per block builds all KB chunk matrices at once via
broadcast access patterns. Per chunk one bf16 TensorE matmul accumulates
agg[dst, f] += sel^T @ et in PSUM. Per block two PE transposes produce
aggT, 4 matmuls apply the GraphConv weight, semantic-attention scores are
reduced locally, one AllReduce combines score sums, and the
softmax-weighted combination is written back transposed; the host
re-transposes and stitches.
"""

import numpy as np
import ml_dtypes

import concourse.mybir as mybir
import concourse.tile as tile
from concourse import bacc
from concourse.bass_utils import run_bass_kernel_spmd

N, F, D, P, E, CORES, SEM_H = 50000, 256, 256, 4, 800000, 8, 128
NC = N // CORES            # 6250 dst nodes per core
CAP = 125                  # nodes per block (3 pad slots each)
NB = NC // CAP             # 50 blocks
NCP = NB * 128             # 6400 padded dst slots per core
FH = F // 128              # 2 feature halves
DH = D // 128              # 2 output halves
GRP = 1                    # dst blocks per et-stream DMA

_nc_cache = {}
_last_in_maps = None


def _n_tiles():
    tiles = []
    off = 0
    while off < NCP:
        t = min(512, NCP - off)
        tiles.append((off, t))
        off += t
    return tiles


def _build(KBs):
    ntiles = _n_tiles()

    nc = bacc.Bacc("TRN2", target_bir_lowering=False, debug=False,
                   num_devices=CORES)
    dt = mybir.dt
    et_in, dst_in = [], []
    for p in range(P):
        CHP = NB * KBs[p]
        et_in.append(nc.dram_tensor(f"et{p}", [128, CHP * F], dt.bfloat16,
                                    kind="ExternalInput").ap())
        dst_in.append(nc.dram_tensor(f"dst{p}", [128, CHP], dt.bfloat16,
                                     kind="ExternalInput").ap())
    Wgc = nc.dram_tensor("Wgc", [P, F, D], dt.float32,
                         kind="ExternalInput").ap()
    W1 = nc.dram_tensor("W1", [D, SEM_H], dt.float32,
                        kind="ExternalInput").ap()
    b1p = nc.dram_tensor("b1p", [SEM_H, P], dt.float32,
                         kind="ExternalInput").ap()
    w2 = nc.dram_tensor("w2", [SEM_H, 1], dt.float32,
                        kind="ExternalInput").ap()
    iota_in = nc.dram_tensor("iota", [128, 128], dt.bfloat16,
                             kind="ExternalInput").ap()
    ident_in = nc.dram_tensor("ident", [128, 128], dt.float32,
                              kind="ExternalInput").ap()
    s4i_in = nc.dram_tensor("s4i", [1, 128], dt.float32,
                            kind="ExternalInput").ap()
    out = nc.dram_tensor("out", [D, NCP], dt.bfloat16,
                         kind="ExternalOutput").ap()

    with tile.TileContext(nc) as tc:
        with (
            tc.tile_pool(name="const", bufs=1) as cp,
            tc.tile_pool(name="stage", bufs=2) as stp,
            tc.tile_pool(name="meta2", bufs=2) as mp2,
            tc.tile_pool(name="edges", bufs=6) as ep,
            tc.tile_pool(name="sel", bufs=4) as selp,
            tc.tile_pool(name="work", bufs=4) as wp,
            tc.tile_pool(name="psum_acc", bufs=3, space="PSUM") as pa,
            tc.tile_pool(name="psum_tp", bufs=2, space="PSUM") as pt,
            tc.tile_pool(name="psum_mm", bufs=2, space="PSUM") as pm,
            tc.tile_pool(name="psum_sp", bufs=1, space="PSUM") as pms,
            tc.tile_pool(name="dram", bufs=2, space="DRAM") as dp,
        ):
            # ---- constants ----
            iota_b = cp.tile([128, 128], dt.bfloat16)
            nc.sync.dma_start(out=iota_b[:], in_=iota_in[:])
            ident = cp.tile([128, 128], dt.float32)
            nc.sync.dma_start(out=ident[:], in_=ident_in[:])
            ones1 = cp.tile([1, 128], dt.float32)
            nc.vector.memset(ones1[:], 1.0)

            dst_sbs = []
            for p in range(P):
                dst_sb = mp2.tile([128, NB * KBs[p]], dt.bfloat16,
                                  tag=f"dst{p}")
                nc.sync.dma_start(out=dst_sb[:], in_=dst_in[p][:])
                dst_sbs.append(dst_sb)

            wgc_sb = []
            for p in range(P):
                per_fh = []
                for fh in range(FH):
                    t32 = stp.tile([128, D], dt.float32)
                    nc.sync.dma_start(out=t32[:],
                                      in_=Wgc[p, fh * 128:(fh + 1) * 128, :])
                    t16 = cp.tile([128, D], dt.bfloat16, name=f"wgc{p}_{fh}")
                    nc.vector.tensor_copy(out=t16[:], in_=t32[:])
                    per_fh.append(t16)
                wgc_sb.append(per_fh)
            w1_sb = []
            for dh in range(DH):
                t32 = stp.tile([128, SEM_H], dt.float32)
                nc.sync.dma_start(out=t32[:],
                                  in_=W1[dh * 128:(dh + 1) * 128, :])
                t16 = cp.tile([128, SEM_H], dt.bfloat16, name=f"w1_{dh}")
                nc.vector.tensor_copy(out=t16[:], in_=t32[:])
                w1_sb.append(t16)
            b1p_sb = cp.tile([128, P], dt.float32)
            nc.sync.dma_start(out=b1p_sb[:], in_=b1p[:])
            w2_32 = stp.tile([128, 1], dt.float32)
            nc.sync.dma_start(out=w2_32[:], in_=w2[:])
            w2_sb = cp.tile([128, 1], dt.bfloat16)
            nc.vector.tensor_copy(out=w2_sb[:], in_=w2_32[:])

            zT_sb = [cp.tile([128, DH * NCP], dt.bfloat16, name=f"zT{p}")
                     for p in range(P)]
            # pad-slot semantic-score correction, precomputed on host
            s4_sb = cp.tile([1, 128], dt.float32)
            nc.sync.dma_start(out=s4_sb[:], in_=s4i_in[:])

            # ---- main: aggregation + per-metapath transform ----
            for p in range(P):
                KB = KBs[p]
                dst_sb = dst_sbs[p]

                def _epilogue(b, agg_tmp):
                    # transposes + GraphConv weight, one block behind the
                    # aggregation matmuls so the PSUM->SBUF copy latency is
                    # hidden under the next block's matmul stream
                    aggT_blk = wp.tile([128, FH * 128], dt.bfloat16,
                                       tag="aggT")
                    for fh in range(FH):
                        tp_ps = pt.tile([128, 128], dt.float32, tag="tpz")
                        nc.tensor.transpose(
                            out=tp_ps[:],
                            in_=agg_tmp[:, fh * 128:(fh + 1) * 128],
                            identity=ident[:])
                        nc.scalar.activation(
                            out=aggT_blk[:, fh * 128:(fh + 1) * 128],
                            in_=tp_ps[:],
                            func=mybir.ActivationFunctionType.Copy)
                    # zT(block) = W^T @ aggT  (bias folded into b1p)
                    for dh in range(DH):
                        zp = pt.tile([128, 128], dt.float32, tag="tpz")
                        for fh in range(FH):
                            nc.tensor.matmul(
                                out=zp[:],
                                lhsT=wgc_sb[p][fh][:,
                                                   dh * 128:(dh + 1) * 128],
                                rhs=aggT_blk[:, fh * 128:(fh + 1) * 128],
                                start=(fh == 0), stop=(fh == FH - 1))
                        nc.scalar.activation(
                            out=zT_sb[p][:, dh * NCP + b * 128:
                                         dh * NCP + (b + 1) * 128],
                            in_=zp[:],
                            func=mybir.ActivationFunctionType.Copy)

                pend = None
                for b in range(NB):
                    et = ep.tile([128, KB * F], dt.bfloat16)
                    nc.sync.dma_start(
                        out=et[:],
                        in_=et_in[p][:, b * KB * F:(b + 1) * KB * F])
                    sel = selp.tile([128, KB * 128], dt.bfloat16)
                    nc.vector.tensor_tensor(
                        out=sel[:].rearrange("q (k j) -> q k j", k=KB),
                        in0=iota_b[:].unsqueeze(1).broadcast_to(
                            (128, KB, 128)),
                        in1=dst_sb[:, b * KB:(b + 1) * KB].unsqueeze(
                            2).broadcast_to((128, KB, 128)),
                        op=mybir.AluOpType.is_equal)
                    acc = pa.tile([128, F], dt.float32, name="acc")
                    for k in range(KB):
                        nc.tensor.matmul(
                            out=acc[:],
                            lhsT=sel[:, k * 128:(k + 1) * 128],
                            rhs=et[:, k * F:(k + 1) * F],
                            start=(k == 0), stop=(k == KB - 1))
                    agg_tmp = wp.tile([128, F], dt.float32, tag="aggtmp")
                    nc.scalar.activation(
                        out=agg_tmp[:], in_=acc[:],
                        func=mybir.ActivationFunctionType.Copy)
                    if pend is not None:
                        _epilogue(*pend)
                    pend = (b, agg_tmp)
                _epilogue(*pend)

                # semantic attention scores: s = tanh(z@W1 + b1') @ w2
                sp_acc = pms.tile([1, 512], dt.float32, tag="spacc")
                for ti, (n0, nt) in enumerate(ntiles):
                    tp = pm.tile([128, 512], dt.float32, tag="mm")
                    for dh in range(DH):
                        nc.tensor.matmul(
                            out=tp[:, :nt],
                            lhsT=w1_sb[dh][:],
                            rhs=zT_sb[p][:, dh * NCP + n0:dh * NCP + n0 + nt],
                            start=(dh == 0), stop=(dh == DH - 1))
                    t_sb = wp.tile([128, 512], dt.bfloat16)
                    nc.scalar.activation(
                        out=t_sb[:, :nt], in_=tp[:, :nt],
                        func=mybir.ActivationFunctionType.Tanh,
                        bias=b1p_sb[:, p:p + 1])
                    nc.tensor.matmul(out=sp_acc[:, :nt], lhsT=w2_sb[:],
                                     rhs=t_sb[:, :nt], start=(ti == 0),
                                     stop=(ti == len(ntiles) - 1))
                stmp = wp.tile([1, 1], dt.float32)
                nc.vector.tensor_reduce(
                    out=stmp[:], in_=sp_acc[:],
                    axis=mybir.AxisListType.X, op=mybir.AluOpType.add)
                nc.vector.tensor_tensor(
                    out=s4_sb[:, p:p + 1], in0=s4_sb[:, p:p + 1],
                    in1=stmp[:], op=mybir.AluOpType.add)

            # ---- semantic softmax over metapaths (global mean via AllReduce)
            cc_in = dp.tile([1, P], dt.float32)
            cc_out = dp.tile([1, P], dt.float32)
            nc.sync.dma_start(out=cc_in[:], in_=s4_sb[:, :P])
            nc.gpsimd.collective_compute(
                "AllReduce", mybir.AluOpType.add,
                replica_groups=[list(range(CORES))],
                ins=[cc_in.opt()], outs=[cc_out.opt()])
            sall = wp.tile([1, P], dt.float32)
            nc.sync.dma_start(out=sall[:], in_=cc_out[:])

            bexp = wp.tile([1, P], dt.float32)
            nc.scalar.activation(out=bexp[:], in_=sall[:, :P],
                                 func=mybir.ActivationFunctionType.Exp,
                                 scale=1.0 / N)
            bsum = wp.tile([1, 1], dt.float32)
            nc.vector.tensor_reduce(out=bsum[:], in_=bexp[:],
                                    axis=mybir.AxisListType.X,
                                    op=mybir.AluOpType.add)
            binv = wp.tile([1, 1], dt.float32)
            nc.vector.reciprocal(out=binv[:], in_=bsum[:])
            bnorm = wp.tile([1, P], dt.float32)
            nc.vector.tensor_scalar_mul(out=bnorm[:], in0=bexp[:],
                                        scalar1=binv[:, 0:1])
            bb_ps = pm.tile([128, P], dt.float32, tag="mm")
            nc.tensor.matmul(out=bb_ps[:], lhsT=ones1[:], rhs=bnorm[:],
                             start=True, stop=True)
            bb_sb = wp.tile([128, P], dt.float32)
            nc.vector.tensor_copy(out=bb_sb[:], in_=bb_ps[:])
            diag = []
            for p in range(P):
                dg = cp.tile([128, 128], dt.bfloat16, name=f"diag{p}")
                nc.vector.tensor_scalar_mul(out=dg[:], in0=ident[:],
                                            scalar1=bb_sb[:, p:p + 1])
                diag.append(dg)

            # ---- weighted combine + output ----
            for dh in range(DH):
                for (n0, nt) in ntiles:
                    op_ps = pm.tile([128, 512], dt.float32, tag="mm")
                    for p in range(P):
                        nc.tensor.matmul(
                            out=op_ps[:, :nt], lhsT=diag[p][:],
                            rhs=zT_sb[p][:, dh * NCP + n0:dh * NCP + n0 + nt],
                            start=(p == 0), stop=(p == P - 1))
                    ot = wp.tile([128, 512], dt.bfloat16)
                    nc.scalar.activation(
                        out=ot[:, :nt], in_=op_ps[:, :nt],
                        func=mybir.ActivationFunctionType.Copy)
                    nc.sync.dma_start(
                        out=out[dh * 128:(dh + 1) * 128, n0:n0 + nt],
                        in_=ot[:, :nt])
    nc.compile()
    return nc


def _balance(deg, caps):
    """Assign NC nodes to NB blocks, balancing all P per-metapath in-degree
    sums simultaneously (greedy, heaviest node first)."""
    order = np.argsort(-deg.sum(axis=0), kind="stable")
    loads = np.zeros((NB, deg.shape[0]), dtype=np.int64)
    counts = np.zeros(NB, dtype=np.int64)
    assign = np.empty(NC, dtype=np.int64)
    for n in order:
        feas = counts < caps
        newmax = np.where(feas[:, None], loads + deg[:, n],
                          1 << 40).max(axis=1)
        b = int(np.argmin(newmax))
        assign[n] = b
        loads[b] += deg[:, n]
        counts[b] += 1
    return assign, loads.max(axis=0)


def _prep_core(h32, src_p, dst_p, w_p, base, KB, blk_of, pos_of):
    """Per-core, per-metapath host staging: returns (et [128, CHP*F] bf16,
    dstpos [128, CHP] bf16) in block-major chunk layout."""
    CHP = NB * KB
    m = (dst_p >= base) & (dst_p < base + NC)
    s, d, w = src_p[m], dst_p[m] - base, w_p[m]
    blk = blk_of[d]
    order = np.argsort(blk, kind="stable")
    s, d, w, blk = s[order], d[order], w[order], blk[order]
    cnt = np.bincount(blk, minlength=NB)
    start = np.concatenate([[0], np.cumsum(cnt)])[:-1]
    r = np.arange(len(d)) - start[blk]          # rank within block
    assert cnt.max() <= KB * 128
    slot = (blk * KB + r // 128) * 128 + r % 128

    dstpos = np.full(CHP * 128, -1.0, dtype=np.float32)
    dstpos[slot] = pos_of[d]
    et = np.zeros((CHP * 128, F), dtype=ml_dtypes.bfloat16)
    et[slot] = (h32[s] * w[:, None]).astype(ml_dtypes.bfloat16)
    et = np.ascontiguousarray(
        et.reshape(CHP, 128, F).transpose(1, 0, 2)).reshape(128, CHP * F)
    dstpos = np.ascontiguousarray(
        dstpos.reshape(CHP, 128).T).astype(ml_dtypes.bfloat16)
    return et, dstpos


def kernel(h, src, dst, W_gc, b_gc, W1, b1, w2):
    h = np.ascontiguousarray(h, dtype=np.float32)
    src = np.asarray(src)
    dst = np.asarray(dst)
    W_gc = np.ascontiguousarray(W_gc, dtype=np.float32)
    b_gc = np.asarray(b_gc, dtype=np.float32)
    W1 = np.ascontiguousarray(W1, dtype=np.float32)
    b1 = np.asarray(b1, dtype=np.float32)
    w2 = np.asarray(w2, dtype=np.float32)

    w_edge = []
    for p in range(P):
        deg_out = np.clip(np.bincount(src[p], minlength=N), 1, None)
        deg_in = np.clip(np.bincount(dst[p], minlength=N), 1, None)
        w_edge.append((1.0 / np.sqrt(deg_out[src[p]]) /
                       np.sqrt(deg_in[dst[p]])).astype(np.float32))

    caps = np.full(NB, CAP, dtype=np.int64)
    blk_of, pos_of = [], []
    maxload = np.zeros(P, dtype=np.int64)
    for c in range(CORES):
        base = c * NC
        degs = []
        for p in range(P):
            m = (dst[p] >= base) & (dst[p] < base + NC)
            degs.append(np.bincount(dst[p][m] - base, minlength=NC))
        assign, mx = _balance(np.stack(degs), caps)
        maxload = np.maximum(maxload, mx)
        order = np.argsort(assign, kind="stable")
        pos = np.empty(NC, dtype=np.int64)
        starts = np.concatenate([[0], np.cumsum(np.bincount(assign,
                                                            minlength=NB))])
        pos[order] = np.arange(NC) - starts[assign[order]]
        blk_of.append(assign)
        pos_of.append(pos.astype(np.float32))
    KBs = tuple(max(1, int(-(-maxload[p] // 128))) for p in range(P))

    if KBs not in _nc_cache:
        _nc_cache[KBs] = _build(KBs)
    nc = _nc_cache[KBs]

    b1p = np.stack([b1 + W1.T @ b_gc[p] for p in range(P)], axis=1)
    iota = np.arange(128, dtype=np.float32)[None, :].repeat(128, axis=0)
    ident = np.eye(128, dtype=np.float32)
    # pad slots contribute tanh(b1p)@w2 each to the per-core score sum
    s4i = np.zeros((1, 128), dtype=np.float32)
    s4i[0, :P] = -(NCP - NC) * (np.tanh(b1p.T.astype(np.float32)) @ w2)

    in_maps = []
    for c in range(CORES):
        base = c * NC
        im = {
            "Wgc": W_gc,
            "W1": W1,
            "b1p": b1p,
            "w2": w2.reshape(SEM_H, 1),
            "iota": iota.astype(ml_dtypes.bfloat16),
            "ident": ident,
            "s4i": s4i,
        }
        for p in range(P):
            et, dstpos = _prep_core(h, src[p], dst[p], w_edge[p], base,
                                    KBs[p], blk_of[c], pos_of[c])
            im[f"et{p}"] = et
            im[f"dst{p}"] = dstpos
        in_maps.append(im)

    global _last_in_maps
    _last_in_maps = in_maps
    res = run_bass_kernel_spmd(nc, in_maps, list(range(CORES))).results
    out = np.empty((N, D), dtype=np.float32)
    for c in range(CORES):
        slot = blk_of[c] * 128 + pos_of[c].astype(np.int64)
        out[c * NC:(c + 1) * NC] = res[c]["out"][:, slot].T.astype(np.float32)
    return out


# revision 21
# speedup vs baseline: 1.1514x; 1.1514x over previous
"""HAN forward on 8 trn2 NeuronCores — host-staged edge-stream version.

Dst nodes sharded across 8 cores (6250 each), assigned to 50 blocks of 125
by a load balancer so per-(block, metapath) edge counts are uniform (16
chunks of 128 edge slots per block). The host materializes, per core and
metapath, the edge message stream et[slot] = h[src] / sqrt(deg_out*deg_in)
in bf16, laid out partition-major ([128 lanes, chunk*256 cols]) so the
device streams it with large fully-contiguous HWDGE DMAs — no on-device
gather (SWDGE descriptor generation on GpSimd was the original
bottleneck). Selection matrices are binary (normalization folded into the
stream on the host): one batched DVE tensor_tensor is_equal per block
builds all 16 chunk matrices at once via broadcast access patterns. Per
chunk one bf16 TensorE matmul accumulates agg[dst, f] += sel^T @ et in
PSUM. Per block two PE transposes produce aggT and 4 matmuls apply the
GraphConv weight. Semantic-attention scores are reduced locally (pad slots
corrected via a host-computed initial value), one AllReduce combines score
sums, and the softmax-weighted combination is written back transposed in
bf16; the host re-transposes and stitches.
"""

import numpy as np
import ml_dtypes

import concourse.mybir as mybir
import concourse.tile as tile
from concourse import bacc
from concourse.bass_utils import run_bass_kernel_spmd

N, F, D, P, E, CORES, SEM_H = 50000, 256, 256, 4, 800000, 8, 128
NC = N // CORES            # 6250 dst nodes per core
CAP = 125                  # nodes per block (3 pad slots each)
NB = NC // CAP             # 50 blocks
NCP = NB * 128             # 6400 padded dst slots per core
FH = F // 128              # 2 feature halves
DH = D // 128              # 2 output halves
GRP = 2                    # dst blocks per et-stream DMA

_nc_cache = {}
_last_in_maps = None


def _n_tiles():
    tiles = []
    off = 0
    while off < NCP:
        t = min(512, NCP - off)
        tiles.append((off, t))
        off += t
    return tiles


def _build(KBs):
    ntiles = _n_tiles()

    nc = bacc.Bacc("TRN2", target_bir_lowering=False, debug=False,
                   num_devices=CORES)
    dt = mybir.dt
    et_in, dst_in = [], []
    for p in range(P):
        CHP = NB * KBs[p]
        et_in.append(nc.dram_tensor(f"et{p}", [128, CHP * F], dt.bfloat16,
                                    kind="ExternalInput").ap())
        dst_in.append(nc.dram_tensor(f"dst{p}", [128, CHP], dt.bfloat16,
                                     kind="ExternalInput").ap())
    Wgc = nc.dram_tensor("Wgc", [P, F, D], dt.float32,
                         kind="ExternalInput").ap()
    W1 = nc.dram_tensor("W1", [D, SEM_H], dt.float32,
                        kind="ExternalInput").ap()
    b1p = nc.dram_tensor("b1p", [SEM_H, P], dt.float32,
                         kind="ExternalInput").ap()
    w2 = nc.dram_tensor("w2", [SEM_H, 1], dt.float32,
                        kind="ExternalInput").ap()
    iota_in = nc.dram_tensor("iota", [128, 128], dt.bfloat16,
                             kind="ExternalInput").ap()
    ident_in = nc.dram_tensor("ident", [128, 128], dt.float32,
                              kind="ExternalInput").ap()
    s4i_in = nc.dram_tensor("s4i", [1, 128], dt.float32,
                            kind="ExternalInput").ap()
    out = nc.dram_tensor("out", [D, NCP], dt.bfloat16,
                         kind="ExternalOutput").ap()

    with tile.TileContext(nc) as tc:
        with (
            tc.tile_pool(name="const", bufs=1) as cp,
            tc.tile_pool(name="stage", bufs=2) as stp,
            tc.tile_pool(name="meta2", bufs=2) as mp2,
            tc.tile_pool(name="edges", bufs=3) as ep,
            tc.tile_pool(name="sel", bufs=4) as selp,
            tc.tile_pool(name="work", bufs=4) as wp,
            tc.tile_pool(name="psum_acc", bufs=3, space="PSUM") as pa,
            tc.tile_pool(name="psum_tp", bufs=2, space="PSUM") as pt,
            tc.tile_pool(name="psum_mm", bufs=3, space="PSUM") as pm,
            tc.tile_pool(name="dram", bufs=2, space="DRAM") as dp,
        ):
            # ---- constants ----
            iota_b = cp.tile([128, 128], dt.bfloat16)
            nc.sync.dma_start(out=iota_b[:], in_=iota_in[:])
            ident = cp.tile([128, 128], dt.float32)
            nc.sync.dma_start(out=ident[:], in_=ident_in[:])
            ones1 = cp.tile([1, 128], dt.float32)
            nc.vector.memset(ones1[:], 1.0)

            wgc_sb = []
            for p in range(P):
                per_fh = []
                for fh in range(FH):
                    t32 = stp.tile([128, D], dt.float32)
                    nc.sync.dma_start(out=t32[:],
                                      in_=Wgc[p, fh * 128:(fh + 1) * 128, :])
                    t16 = cp.tile([128, D], dt.bfloat16, name=f"wgc{p}_{fh}")
                    nc.vector.tensor_copy(out=t16[:], in_=t32[:])
                    per_fh.append(t16)
                wgc_sb.append(per_fh)
            w1_sb = []
            for dh in range(DH):
                t32 = stp.tile([128, SEM_H], dt.float32)
                nc.sync.dma_start(out=t32[:],
                                  in_=W1[dh * 128:(dh + 1) * 128, :])
                t16 = cp.tile([128, SEM_H], dt.bfloat16, name=f"w1_{dh}")
                nc.vector.tensor_copy(out=t16[:], in_=t32[:])
                w1_sb.append(t16)
            b1p_sb = cp.tile([128, P], dt.float32)
            nc.sync.dma_start(out=b1p_sb[:], in_=b1p[:])
            w2_32 = stp.tile([128, 1], dt.float32)
            nc.sync.dma_start(out=w2_32[:], in_=w2[:])
            w2_sb = cp.tile([128, 1], dt.bfloat16)
            nc.vector.tensor_copy(out=w2_sb[:], in_=w2_32[:])

            zT_sb = [cp.tile([128, DH * NCP], dt.bfloat16, name=f"zT{p}")
                     for p in range(P)]
            # pad-slot semantic-score correction, precomputed on host
            s4_sb = cp.tile([1, 128], dt.float32)
            nc.sync.dma_start(out=s4_sb[:], in_=s4i_in[:])

            dst_sbs = []
            for p in range(P):
                dst_sb = mp2.tile([128, NB * KBs[p]], dt.bfloat16,
                                  tag=f"dst{p}")
                nc.sync.dma_start(out=dst_sb[:], in_=dst_in[p][:])
                dst_sbs.append(dst_sb)

            # ---- main: aggregation + per-metapath transform ----
            for p in range(P):
                KB = KBs[p]
                dst_sb = dst_sbs[p]

                b0 = 0
                while b0 < NB:
                    ng = min(GRP, NB - b0)
                    et = ep.tile([128, GRP * KB * F], dt.bfloat16)
                    nc.sync.dma_start(
                        out=et[:, :ng * KB * F],
                        in_=et_in[p][:, b0 * KB * F:(b0 + ng) * KB * F])
                    for i in range(ng):
                        b = b0 + i
                        sel = selp.tile([128, KB * 128], dt.bfloat16)
                        nc.vector.tensor_tensor(
                            out=sel[:].rearrange("q (k j) -> q k j", k=KB),
                            in0=iota_b[:].unsqueeze(1).broadcast_to(
                                (128, KB, 128)),
                            in1=dst_sb[:, b * KB:(b + 1) * KB].unsqueeze(
                                2).broadcast_to((128, KB, 128)),
                            op=mybir.AluOpType.is_equal)
                        acc = pa.tile([128, F], dt.float32, name="acc")
                        for k in range(KB):
                            nc.tensor.matmul(
                                out=acc[:],
                                lhsT=sel[:, k * 128:(k + 1) * 128],
                                rhs=et[:, (i * KB + k) * F:
                                       (i * KB + k + 1) * F],
                                start=(k == 0), stop=(k == KB - 1))
                        agg_tmp = wp.tile([128, F], dt.float32, tag="aggtmp")
                        nc.scalar.activation(
                            out=agg_tmp[:], in_=acc[:],
                            func=mybir.ActivationFunctionType.Copy)
                        aggT_blk = wp.tile([128, FH * 128], dt.bfloat16,
                                           tag="aggT")
                        for fh in range(FH):
                            tp_ps = pt.tile([128, 128], dt.float32, tag="tpz")
                            nc.tensor.transpose(
                                out=tp_ps[:],
                                in_=agg_tmp[:, fh * 128:(fh + 1) * 128],
                                identity=ident[:])
                            nc.scalar.activation(
                                out=aggT_blk[:, fh * 128:(fh + 1) * 128],
                                in_=tp_ps[:],
                                func=mybir.ActivationFunctionType.Copy)
                        # zT(block) = W^T @ aggT  (bias folded into b1p)
                        for dh in range(DH):
                            zp = pt.tile([128, 128], dt.float32, tag="tpz")
                            for fh in range(FH):
                                nc.tensor.matmul(
                                    out=zp[:],
                                    lhsT=wgc_sb[p][fh][:,
                                                       dh * 128:
                                                       (dh + 1) * 128],
                                    rhs=aggT_blk[:, fh * 128:(fh + 1) * 128],
                                    start=(fh == 0), stop=(fh == FH - 1))
                            nc.scalar.activation(
                                out=zT_sb[p][:, dh * NCP + b * 128:
                                             dh * NCP + (b + 1) * 128],
                                in_=zp[:],
                                func=mybir.ActivationFunctionType.Copy)
                    b0 += ng

                # semantic attention scores: s = tanh(z@W1 + b1') @ w2
                for (n0, nt) in ntiles:
                    tp = pm.tile([128, 512], dt.float32, tag="mm")
                    for dh in range(DH):
                        nc.tensor.matmul(
                            out=tp[:, :nt],
                            lhsT=w1_sb[dh][:],
                            rhs=zT_sb[p][:, dh * NCP + n0:dh * NCP + n0 + nt],
                            start=(dh == 0), stop=(dh == DH - 1))
                    t_sb = wp.tile([128, 512], dt.bfloat16)
                    nc.scalar.activation(
                        out=t_sb[:, :nt], in_=tp[:, :nt],
                        func=mybir.ActivationFunctionType.Tanh,
                        bias=b1p_sb[:, p:p + 1])
                    sp = pm.tile([1, 512], dt.float32, tag="mm")
                    nc.tensor.matmul(out=sp[:, :nt], lhsT=w2_sb[:],
                                     rhs=t_sb[:, :nt], start=True, stop=True)
                    stmp = wp.tile([1, 1], dt.float32)
                    nc.vector.tensor_reduce(
                        out=stmp[:], in_=sp[:, :nt],
                        axis=mybir.AxisListType.X, op=mybir.AluOpType.add)
                    nc.vector.tensor_tensor(
                        out=s4_sb[:, p:p + 1], in0=s4_sb[:, p:p + 1],
                        in1=stmp[:], op=mybir.AluOpType.add)

            # ---- semantic softmax over metapaths (global mean via AllReduce)
            cc_in = dp.tile([1, 128], dt.float32)
            cc_out = dp.tile([1, 128], dt.float32)
            nc.sync.dma_start(out=cc_in[:], in_=s4_sb[:])
            nc.gpsimd.collective_compute(
                "AllReduce", mybir.AluOpType.add,
                replica_groups=[list(range(CORES))],
                ins=[cc_in.opt()], outs=[cc_out.opt()])
            sall = wp.tile([1, 128], dt.float32)
            nc.sync.dma_start(out=sall[:], in_=cc_out[:])

            bexp = wp.tile([1, P], dt.float32)
            nc.scalar.activation(out=bexp[:], in_=sall[:, :P],
                                 func=mybir.ActivationFunctionType.Exp,
                                 scale=1.0 / N)
            bsum = wp.tile([1, 1], dt.float32)
            nc.vector.tensor_reduce(out=bsum[:], in_=bexp[:],
                                    axis=mybir.AxisListType.X,
                                    op=mybir.AluOpType.add)
            binv = wp.tile([1, 1], dt.float32)
            nc.vector.reciprocal(out=binv[:], in_=bsum[:])
            bnorm = wp.tile([1, P], dt.float32)
            nc.vector.tensor_scalar_mul(out=bnorm[:], in0=bexp[:],
                                        scalar1=binv[:, 0:1])
            bb_ps = pm.tile([128, P], dt.float32, tag="mm")
            nc.tensor.matmul(out=bb_ps[:], lhsT=ones1[:], rhs=bnorm[:],
                             start=True, stop=True)
            bb_sb = wp.tile([128, P], dt.float32)
            nc.vector.tensor_copy(out=bb_sb[:], in_=bb_ps[:])
            diag = []
            for p in range(P):
                dg = cp.tile([128, 128], dt.bfloat16, name=f"diag{p}")
                nc.vector.tensor_scalar_mul(out=dg[:], in0=ident[:],
                                            scalar1=bb_sb[:, p:p + 1])
                diag.append(dg)

            # ---- weighted combine + output ----
            for dh in range(DH):
                for (n0, nt) in ntiles:
                    op_ps = pm.tile([128, 512], dt.float32, tag="mm")
                    for p in range(P):
                        nc.tensor.matmul(
                            out=op_ps[:, :nt], lhsT=diag[p][:],
                            rhs=zT_sb[p][:, dh * NCP + n0:dh * NCP + n0 + nt],
                            start=(p == 0), stop=(p == P - 1))
                    ot = wp.tile([128, 512], dt.bfloat16)
                    nc.vector.tensor_copy(out=ot[:, :nt], in_=op_ps[:, :nt])
                    nc.sync.dma_start(
                        out=out[dh * 128:(dh + 1) * 128, n0:n0 + nt],
                        in_=ot[:, :nt])
    nc.compile()
    return nc


def _balance(deg, caps):
    """Assign NC nodes to NB blocks, balancing all P per-metapath in-degree
    sums simultaneously (greedy, heaviest node first)."""
    order = np.argsort(-deg.sum(axis=0), kind="stable")
    loads = np.zeros((NB, deg.shape[0]), dtype=np.int64)
    counts = np.zeros(NB, dtype=np.int64)
    assign = np.empty(NC, dtype=np.int64)
    for n in order:
        feas = counts < caps
        newmax = np.where(feas[:, None], loads + deg[:, n],
                          1 << 40).max(axis=1)
        b = int(np.argmin(newmax))
        assign[n] = b
        loads[b] += deg[:, n]
        counts[b] += 1
    return assign, loads.max(axis=0)


def _prep_core(h32, src_p, dst_p, w_p, base, KB, blk_of, pos_of):
    """Per-core, per-metapath host staging: returns (et [128, CHP*F] bf16,
    dstpos [128, CHP] bf16) in block-major chunk layout."""
    CHP = NB * KB
    m = (dst_p >= base) & (dst_p < base + NC)
    s, d, w = src_p[m], dst_p[m] - base, w_p[m]
    blk = blk_of[d]
    order = np.argsort(blk, kind="stable")
    s, d, w, blk = s[order], d[order], w[order], blk[order]
    cnt = np.bincount(blk, minlength=NB)
    start = np.concatenate([[0], np.cumsum(cnt)])[:-1]
    r = np.arange(len(d)) - start[blk]          # rank within block
    assert cnt.max() <= KB * 128
    slot = (blk * KB + r // 128) * 128 + r % 128

    dstpos = np.full(CHP * 128, -1.0, dtype=np.float32)
    dstpos[slot] = pos_of[d]
    et = np.zeros((CHP * 128, F), dtype=ml_dtypes.bfloat16)
    et[slot] = (h32[s] * w[:, None]).astype(ml_dtypes.bfloat16)
    et = np.ascontiguousarray(
        et.reshape(CHP, 128, F).transpose(1, 0, 2)).reshape(128, CHP * F)
    dstpos = np.ascontiguousarray(
        dstpos.reshape(CHP, 128).T).astype(ml_dtypes.bfloat16)
    return et, dstpos


def kernel(h, src, dst, W_gc, b_gc, W1, b1, w2):
    h = np.ascontiguousarray(h, dtype=np.float32)
    src = np.asarray(src)
    dst = np.asarray(dst)
    W_gc = np.ascontiguousarray(W_gc, dtype=np.float32)
    b_gc = np.asarray(b_gc, dtype=np.float32)
    W1 = np.ascontiguousarray(W1, dtype=np.float32)
    b1 = np.asarray(b1, dtype=np.float32)
    w2 = np.asarray(w2, dtype=np.float32)

    w_edge = []
    for p in range(P):
        deg_out = np.clip(np.bincount(src[p], minlength=N), 1, None)
        deg_in = np.clip(np.bincount(dst[p], minlength=N), 1, None)
        w_edge.append((1.0 / np.sqrt(deg_out[src[p]]) /
                       np.sqrt(deg_in[dst[p]])).astype(np.float32))

    caps = np.full(NB, CAP, dtype=np.int64)
    blk_of, pos_of = [], []
    maxload = np.zeros(P, dtype=np.int64)
    for c in range(CORES):
        base = c * NC
        degs = []
        for p in range(P):
            m = (dst[p] >= base) & (dst[p] < base + NC)
            degs.append(np.bincount(dst[p][m] - base, minlength=NC))
        assign, mx = _balance(np.stack(degs), caps)
        maxload = np.maximum(maxload, mx)
        order = np.argsort(assign, kind="stable")
        pos = np.empty(NC, dtype=np.int64)
        starts = np.concatenate([[0], np.cumsum(np.bincount(assign,
                                                            minlength=NB))])
        pos[order] = np.arange(NC) - starts[assign[order]]
        blk_of.append(assign)
        pos_of.append(pos.astype(np.float32))
    KBs = tuple(max(1, int(-(-maxload[p] // 128))) for p in range(P))

    if KBs not in _nc_cache:
        _nc_cache[KBs] = _build(KBs)
    nc = _nc_cache[KBs]

    b1p = np.stack([b1 + W1.T @ b_gc[p] for p in range(P)], axis=1)
    iota = np.arange(128, dtype=np.float32)[None, :].repeat(128, axis=0)
    ident = np.eye(128, dtype=np.float32)
    # pad slots contribute tanh(b1p)@w2 each to the per-core score sum
    s4i = np.zeros((1, 128), dtype=np.float32)
    s4i[0, :P] = -(NCP - NC) * (np.tanh(b1p.T.astype(np.float32)) @ w2)

    in_maps = []
    for c in range(CORES):
        base = c * NC
        im = {
            "Wgc": W_gc,
            "W1": W1,
            "b1p": b1p,
            "w2": w2.reshape(SEM_H, 1),
            "iota": iota.astype(ml_dtypes.bfloat16),
            "ident": ident,
            "s4i": s4i,
        }
        for p in range(P):
            et, dstpos = _prep_core(h, src[p], dst[p], w_edge[p], base,
                                    KBs[p], blk_of[c], pos_of[c])
            im[f"et{p}"] = et
            im[f"dst{p}"] = dstpos
        in_maps.append(im)

    global _last_in_maps
    _last_in_maps = in_maps
    res = run_bass_kernel_spmd(nc, in_maps, list(range(CORES))).results
    out = np.empty((N, D), dtype=np.float32)
    for c in range(CORES):
        slot = blk_of[c] * 128 + pos_of[c].astype(np.int64)
        out[c * NC:(c + 1) * NC] = res[c]["out"][:, slot].T.astype(np.float32)
    return out


# revision 22
# speedup vs baseline: 1.1547x; 1.0029x over previous
"""HAN forward on 8 trn2 NeuronCores — host-staged edge-stream version.

Dst nodes sharded across 8 cores (6250 each), assigned to 50 blocks of 125
by a load balancer so per-(block, metapath) edge counts are uniform (16
chunks of 128 edge slots per block). The host materializes, per core and
metapath, the edge message stream et[slot] = h[src] / sqrt(deg_out*deg_in)
in bf16, laid out partition-major ([128 lanes, chunk*256 cols]) so the
device streams it with large fully-contiguous HWDGE DMAs — no on-device
gather (SWDGE descriptor generation on GpSimd was the original
bottleneck). Selection matrices are binary (normalization folded into the
stream on the host): one batched DVE tensor_tensor is_equal per block
builds all 16 chunk matrices at once via broadcast access patterns. Per
chunk one bf16 TensorE matmul accumulates agg[dst, f] += sel^T @ et in
PSUM. Per block two PE transposes produce aggT and 4 matmuls apply the
GraphConv weight. Semantic-attention scores are reduced locally (pad slots
corrected via a host-computed initial value), one AllReduce combines score
sums, and the softmax-weighted combination is written back transposed in
bf16; the host re-transposes and stitches.
"""

import numpy as np
import ml_dtypes

import concourse.mybir as mybir
import concourse.tile as tile
from concourse import bacc
from concourse.bass_utils import run_bass_kernel_spmd

N, F, D, P, E, CORES, SEM_H = 50000, 256, 256, 4, 800000, 8, 128
NC = N // CORES            # 6250 dst nodes per core
CAP = 125                  # nodes per block (3 pad slots each)
NB = NC // CAP             # 50 blocks
NCP = NB * 128             # 6400 padded dst slots per core
FH = F // 128              # 2 feature halves
DH = D // 128              # 2 output halves
GRP = 2                    # dst blocks per et-stream DMA

_nc_cache = {}
_last_in_maps = None


def _n_tiles():
    tiles = []
    off = 0
    while off < NCP:
        t = min(512, NCP - off)
        tiles.append((off, t))
        off += t
    return tiles


def _build(KBs):
    ntiles = _n_tiles()

    nc = bacc.Bacc("TRN2", target_bir_lowering=False, debug=False,
                   num_devices=CORES)
    dt = mybir.dt
    et_in, dst_in = [], []
    for p in range(P):
        CHP = NB * KBs[p]
        et_in.append(nc.dram_tensor(f"et{p}", [128, CHP * F], dt.bfloat16,
                                    kind="ExternalInput").ap())
        dst_in.append(nc.dram_tensor(f"dst{p}", [128, CHP], dt.bfloat16,
                                     kind="ExternalInput").ap())
    Wgc = nc.dram_tensor("Wgc", [P, F, D], dt.float32,
                         kind="ExternalInput").ap()
    W1 = nc.dram_tensor("W1", [D, SEM_H], dt.float32,
                        kind="ExternalInput").ap()
    b1p = nc.dram_tensor("b1p", [SEM_H, P], dt.float32,
                         kind="ExternalInput").ap()
    w2 = nc.dram_tensor("w2", [SEM_H, 1], dt.float32,
                        kind="ExternalInput").ap()
    iota_in = nc.dram_tensor("iota", [128, 128], dt.bfloat16,
                             kind="ExternalInput").ap()
    ident_in = nc.dram_tensor("ident", [128, 128], dt.float32,
                              kind="ExternalInput").ap()
    s4i_in = nc.dram_tensor("s4i", [1, 128], dt.float32,
                            kind="ExternalInput").ap()
    out = nc.dram_tensor("out", [D, NCP], dt.bfloat16,
                         kind="ExternalOutput").ap()

    with tile.TileContext(nc) as tc:
        with (
            tc.tile_pool(name="const", bufs=1) as cp,
            tc.tile_pool(name="stage", bufs=2) as stp,
            tc.tile_pool(name="meta2", bufs=2) as mp2,
            tc.tile_pool(name="edges", bufs=3) as ep,
            tc.tile_pool(name="sel", bufs=4) as selp,
            tc.tile_pool(name="work", bufs=4) as wp,
            tc.tile_pool(name="psum_acc", bufs=3, space="PSUM") as pa,
            tc.tile_pool(name="psum_tp", bufs=2, space="PSUM") as pt,
            tc.tile_pool(name="psum_mm", bufs=3, space="PSUM") as pm,
            tc.tile_pool(name="dram", bufs=2, space="DRAM") as dp,
        ):
            # ---- constants ----
            iota_b = cp.tile([128, 128], dt.bfloat16)
            nc.sync.dma_start(out=iota_b[:], in_=iota_in[:])
            ident = cp.tile([128, 128], dt.float32)
            nc.sync.dma_start(out=ident[:], in_=ident_in[:])
            ones1 = cp.tile([1, 128], dt.float32)
            nc.vector.memset(ones1[:], 1.0)

            wgc_sb = []
            for p in range(P):
                per_fh = []
                for fh in range(FH):
                    t32 = stp.tile([128, D], dt.float32)
                    nc.sync.dma_start(out=t32[:],
                                      in_=Wgc[p, fh * 128:(fh + 1) * 128, :])
                    t16 = cp.tile([128, D], dt.bfloat16, name=f"wgc{p}_{fh}")
                    nc.vector.tensor_copy(out=t16[:], in_=t32[:])
                    per_fh.append(t16)
                wgc_sb.append(per_fh)
            w1_sb = []
            for dh in range(DH):
                t32 = stp.tile([128, SEM_H], dt.float32)
                nc.sync.dma_start(out=t32[:],
                                  in_=W1[dh * 128:(dh + 1) * 128, :])
                t16 = cp.tile([128, SEM_H], dt.bfloat16, name=f"w1_{dh}")
                nc.vector.tensor_copy(out=t16[:], in_=t32[:])
                w1_sb.append(t16)
            b1p_sb = cp.tile([128, P], dt.float32)
            nc.sync.dma_start(out=b1p_sb[:], in_=b1p[:])
            w2_32 = stp.tile([128, 1], dt.float32)
            nc.sync.dma_start(out=w2_32[:], in_=w2[:])
            w2_sb = cp.tile([128, 1], dt.bfloat16)
            nc.vector.tensor_copy(out=w2_sb[:], in_=w2_32[:])

            zT_sb = [cp.tile([128, DH * NCP], dt.bfloat16, name=f"zT{p}")
                     for p in range(P)]
            # pad-slot semantic-score correction, precomputed on host
            s4_sb = cp.tile([1, 128], dt.float32)
            nc.sync.dma_start(out=s4_sb[:], in_=s4i_in[:])

            dst_sbs = []
            for p in range(P):
                dst_sb = mp2.tile([128, NB * KBs[p]], dt.bfloat16,
                                  tag=f"dst{p}")
                nc.sync.dma_start(out=dst_sb[:], in_=dst_in[p][:])
                dst_sbs.append(dst_sb)

            # ---- main: aggregation + per-metapath transform ----
            for p in range(P):
                KB = KBs[p]
                dst_sb = dst_sbs[p]

                def _epilogue(b, agg_tmp):
                    # transposes + GraphConv weight, deferred one block so
                    # the PSUM->SBUF copy latency hides under the next
                    # block's aggregation matmul stream
                    aggT_blk = wp.tile([128, FH * 128], dt.bfloat16,
                                       tag="aggT")
                    for fh in range(FH):
                        tp_ps = pt.tile([128, 128], dt.float32, tag="tpz")
                        nc.tensor.transpose(
                            out=tp_ps[:],
                            in_=agg_tmp[:, fh * 128:(fh + 1) * 128],
                            identity=ident[:])
                        nc.scalar.activation(
                            out=aggT_blk[:, fh * 128:(fh + 1) * 128],
                            in_=tp_ps[:],
                            func=mybir.ActivationFunctionType.Copy)
                    # zT(block) = W^T @ aggT  (bias folded into b1p)
                    for dh in range(DH):
                        zp = pt.tile([128, 128], dt.float32, tag="tpz")
                        for fh in range(FH):
                            nc.tensor.matmul(
                                out=zp[:],
                                lhsT=wgc_sb[p][fh][:,
                                                   dh * 128:(dh + 1) * 128],
                                rhs=aggT_blk[:, fh * 128:(fh + 1) * 128],
                                start=(fh == 0), stop=(fh == FH - 1))
                        nc.scalar.activation(
                            out=zT_sb[p][:, dh * NCP + b * 128:
                                         dh * NCP + (b + 1) * 128],
                            in_=zp[:],
                            func=mybir.ActivationFunctionType.Copy)

                pend = None
                b0 = 0
                while b0 < NB:
                    ng = min(GRP, NB - b0)
                    et = ep.tile([128, GRP * KB * F], dt.bfloat16)
                    nc.sync.dma_start(
                        out=et[:, :ng * KB * F],
                        in_=et_in[p][:, b0 * KB * F:(b0 + ng) * KB * F])
                    for i in range(ng):
                        b = b0 + i
                        sel = selp.tile([128, KB * 128], dt.bfloat16)
                        nc.vector.tensor_tensor(
                            out=sel[:].rearrange("q (k j) -> q k j", k=KB),
                            in0=iota_b[:].unsqueeze(1).broadcast_to(
                                (128, KB, 128)),
                            in1=dst_sb[:, b * KB:(b + 1) * KB].unsqueeze(
                                2).broadcast_to((128, KB, 128)),
                            op=mybir.AluOpType.is_equal)
                        acc = pa.tile([128, F], dt.float32, name="acc")
                        for k in range(KB):
                            nc.tensor.matmul(
                                out=acc[:],
                                lhsT=sel[:, k * 128:(k + 1) * 128],
                                rhs=et[:, (i * KB + k) * F:
                                       (i * KB + k + 1) * F],
                                start=(k == 0), stop=(k == KB - 1))
                        agg_tmp = wp.tile([128, F], dt.float32, tag="aggtmp")
                        nc.scalar.activation(
                            out=agg_tmp[:], in_=acc[:],
                            func=mybir.ActivationFunctionType.Copy)
                        if pend is not None:
                            _epilogue(*pend)
                        pend = (b, agg_tmp)
                    b0 += ng
                _epilogue(*pend)

                # semantic attention scores: s = tanh(z@W1 + b1') @ w2
                for (n0, nt) in ntiles:
                    tp = pm.tile([128, 512], dt.float32, tag="mm")
                    for dh in range(DH):
                        nc.tensor.matmul(
                            out=tp[:, :nt],
                            lhsT=w1_sb[dh][:],
                            rhs=zT_sb[p][:, dh * NCP + n0:dh * NCP + n0 + nt],
                            start=(dh == 0), stop=(dh == DH - 1))
                    t_sb = wp.tile([128, 512], dt.bfloat16)
                    nc.scalar.activation(
                        out=t_sb[:, :nt], in_=tp[:, :nt],
                        func=mybir.ActivationFunctionType.Tanh,
                        bias=b1p_sb[:, p:p + 1])
                    sp = pm.tile([1, 512], dt.float32, tag="mm")
                    nc.tensor.matmul(out=sp[:, :nt], lhsT=w2_sb[:],
                                     rhs=t_sb[:, :nt], start=True, stop=True)
                    stmp = wp.tile([1, 1], dt.float32)
                    nc.vector.tensor_reduce(
                        out=stmp[:], in_=sp[:, :nt],
                        axis=mybir.AxisListType.X, op=mybir.AluOpType.add)
                    nc.vector.tensor_tensor(
                        out=s4_sb[:, p:p + 1], in0=s4_sb[:, p:p + 1],
                        in1=stmp[:], op=mybir.AluOpType.add)

            # ---- semantic softmax over metapaths (global mean via AllReduce)
            cc_in = dp.tile([1, 128], dt.float32)
            cc_out = dp.tile([1, 128], dt.float32)
            nc.sync.dma_start(out=cc_in[:], in_=s4_sb[:])
            nc.gpsimd.collective_compute(
                "AllReduce", mybir.AluOpType.add,
                replica_groups=[list(range(CORES))],
                ins=[cc_in.opt()], outs=[cc_out.opt()])
            sall = wp.tile([1, 128], dt.float32)
            nc.sync.dma_start(out=sall[:], in_=cc_out[:])

            bexp = wp.tile([1, P], dt.float32)
            nc.scalar.activation(out=bexp[:], in_=sall[:, :P],
                                 func=mybir.ActivationFunctionType.Exp,
                                 scale=1.0 / N)
            bsum = wp.tile([1, 1], dt.float32)
            nc.vector.tensor_reduce(out=bsum[:], in_=bexp[:],
                                    axis=mybir.AxisListType.X,
                                    op=mybir.AluOpType.add)
            binv = wp.tile([1, 1], dt.float32)
            nc.vector.reciprocal(out=binv[:], in_=bsum[:])
            bnorm = wp.tile([1, P], dt.float32)
            nc.vector.tensor_scalar_mul(out=bnorm[:], in0=bexp[:],
                                        scalar1=binv[:, 0:1])
            bb_ps = pm.tile([128, P], dt.float32, tag="mm")
            nc.tensor.matmul(out=bb_ps[:], lhsT=ones1[:], rhs=bnorm[:],
                             start=True, stop=True)
            bb_sb = wp.tile([128, P], dt.float32)
            nc.vector.tensor_copy(out=bb_sb[:], in_=bb_ps[:])
            diag = []
            for p in range(P):
                dg = cp.tile([128, 128], dt.bfloat16, name=f"diag{p}")
                nc.vector.tensor_scalar_mul(out=dg[:], in0=ident[:],
                                            scalar1=bb_sb[:, p:p + 1])
                diag.append(dg)

            # ---- weighted combine + output ----
            for dh in range(DH):
                for (n0, nt) in ntiles:
                    op_ps = pm.tile([128, 512], dt.float32, tag="mm")
                    for p in range(P):
                        nc.tensor.matmul(
                            out=op_ps[:, :nt], lhsT=diag[p][:],
                            rhs=zT_sb[p][:, dh * NCP + n0:dh * NCP + n0 + nt],
                            start=(p == 0), stop=(p == P - 1))
                    ot = wp.tile([128, 512], dt.bfloat16)
                    nc.vector.tensor_copy(out=ot[:, :nt], in_=op_ps[:, :nt])
                    nc.sync.dma_start(
                        out=out[dh * 128:(dh + 1) * 128, n0:n0 + nt],
                        in_=ot[:, :nt])
    nc.compile()
    return nc


def _balance(deg, caps):
    """Assign NC nodes to NB blocks, balancing all P per-metapath in-degree
    sums simultaneously (greedy, heaviest node first)."""
    order = np.argsort(-deg.sum(axis=0), kind="stable")
    loads = np.zeros((NB, deg.shape[0]), dtype=np.int64)
    counts = np.zeros(NB, dtype=np.int64)
    assign = np.empty(NC, dtype=np.int64)
    for n in order:
        feas = counts < caps
        newmax = np.where(feas[:, None], loads + deg[:, n],
                          1 << 40).max(axis=1)
        b = int(np.argmin(newmax))
        assign[n] = b
        loads[b] += deg[:, n]
        counts[b] += 1
    return assign, loads.max(axis=0)


def _prep_core(h32, src_p, dst_p, w_p, base, KB, blk_of, pos_of):
    """Per-core, per-metapath host staging: returns (et [128, CHP*F] bf16,
    dstpos [128, CHP] bf16) in block-major chunk layout."""
    CHP = NB * KB
    m = (dst_p >= base) & (dst_p < base + NC)
    s, d, w = src_p[m], dst_p[m] - base, w_p[m]
    blk = blk_of[d]
    order = np.argsort(blk, kind="stable")
    s, d, w, blk = s[order], d[order], w[order], blk[order]
    cnt = np.bincount(blk, minlength=NB)
    start = np.concatenate([[0], np.cumsum(cnt)])[:-1]
    r = np.arange(len(d)) - start[blk]          # rank within block
    assert cnt.max() <= KB * 128
    slot = (blk * KB + r // 128) * 128 + r % 128

    dstpos = np.full(CHP * 128, -1.0, dtype=np.float32)
    dstpos[slot] = pos_of[d]
    et = np.zeros((CHP * 128, F), dtype=ml_dtypes.bfloat16)
    et[slot] = (h32[s] * w[:, None]).astype(ml_dtypes.bfloat16)
    et = np.ascontiguousarray(
        et.reshape(CHP, 128, F).transpose(1, 0, 2)).reshape(128, CHP * F)
    dstpos = np.ascontiguousarray(
        dstpos.reshape(CHP, 128).T).astype(ml_dtypes.bfloat16)
    return et, dstpos


def kernel(h, src, dst, W_gc, b_gc, W1, b1, w2):
    h = np.ascontiguousarray(h, dtype=np.float32)
    src = np.asarray(src)
    dst = np.asarray(dst)
    W_gc = np.ascontiguousarray(W_gc, dtype=np.float32)
    b_gc = np.asarray(b_gc, dtype=np.float32)
    W1 = np.ascontiguousarray(W1, dtype=np.float32)
    b1 = np.asarray(b1, dtype=np.float32)
    w2 = np.asarray(w2, dtype=np.float32)

    w_edge = []
    for p in range(P):
        deg_out = np.clip(np.bincount(src[p], minlength=N), 1, None)
        deg_in = np.clip(np.bincount(dst[p], minlength=N), 1, None)
        w_edge.append((1.0 / np.sqrt(deg_out[src[p]]) /
                       np.sqrt(deg_in[dst[p]])).astype(np.float32))

    caps = np.full(NB, CAP, dtype=np.int64)
    blk_of, pos_of = [], []
    maxload = np.zeros(P, dtype=np.int64)
    for c in range(CORES):
        base = c * NC
        degs = []
        for p in range(P):
            m = (dst[p] >= base) & (dst[p] < base + NC)
            degs.append(np.bincount(dst[p][m] - base, minlength=NC))
        assign, mx = _balance(np.stack(degs), caps)
        maxload = np.maximum(maxload, mx)
        order = np.argsort(assign, kind="stable")
        pos = np.empty(NC, dtype=np.int64)
        starts = np.concatenate([[0], np.cumsum(np.bincount(assign,
                                                            minlength=NB))])
        pos[order] = np.arange(NC) - starts[assign[order]]
        blk_of.append(assign)
        pos_of.append(pos.astype(np.float32))
    KBs = tuple(max(1, int(-(-maxload[p] // 128))) for p in range(P))

    if KBs not in _nc_cache:
        _nc_cache[KBs] = _build(KBs)
    nc = _nc_cache[KBs]

    b1p = np.stack([b1 + W1.T @ b_gc[p] for p in range(P)], axis=1)
    iota = np.arange(128, dtype=np.float32)[None, :].repeat(128, axis=0)
    ident = np.eye(128, dtype=np.float32)
    # pad slots contribute tanh(b1p)@w2 each to the per-core score sum
    s4i = np.zeros((1, 128), dtype=np.float32)
    s4i[0, :P] = -(NCP - NC) * (np.tanh(b1p.T.astype(np.float32)) @ w2)

    in_maps = []
    for c in range(CORES):
        base = c * NC
        im = {
            "Wgc": W_gc,
            "W1": W1,
            "b1p": b1p,
            "w2": w2.reshape(SEM_H, 1),
            "iota": iota.astype(ml_dtypes.bfloat16),
            "ident": ident,
            "s4i": s4i,
        }
        for p in range(P):
            et, dstpos = _prep_core(h, src[p], dst[p], w_edge[p], base,
                                    KBs[p], blk_of[c], pos_of[c])
            im[f"et{p}"] = et
            im[f"dst{p}"] = dstpos
        in_maps.append(im)

    global _last_in_maps
    _last_in_maps = in_maps
    res = run_bass_kernel_spmd(nc, in_maps, list(range(CORES))).results
    out = np.empty((N, D), dtype=np.float32)
    for c in range(CORES):
        slot = blk_of[c] * 128 + pos_of[c].astype(np.int64)
        out[c * NC:(c + 1) * NC] = res[c]["out"][:, slot].T.astype(np.float32)
    return out


# revision 25
# speedup vs baseline: 1.2360x; 1.0704x over previous
"""HAN forward on 8 trn2 NeuronCores — host-staged edge-stream version.

Dst nodes sharded across 8 cores (6250 each), assigned to 50 blocks of 125
by a load balancer so per-(block, metapath) edge counts are uniform (16
chunks of 128 edge slots per block). The host materializes, per core and
metapath, the edge message stream et[slot] = h[src] / sqrt(deg_out*deg_in)
in bf16, laid out partition-major ([128 lanes, chunk*256 cols]) so the
device streams it with large fully-contiguous HWDGE DMAs — no on-device
gather (SWDGE descriptor generation on GpSimd was the original
bottleneck). Selection matrices are binary (normalization folded into the
stream on the host): one batched DVE tensor_tensor is_equal per block
builds all 16 chunk matrices at once via broadcast access patterns. Per
chunk one bf16 TensorE matmul accumulates agg[dst, f] += sel^T @ et in
PSUM. Per block two PE transposes produce aggT and 4 matmuls apply the
GraphConv weight. Semantic-attention scores are reduced locally (pad slots
corrected via a host-computed initial value), one AllReduce combines score
sums, and the softmax-weighted combination is written back transposed in
bf16; the host re-transposes and stitches.
"""

import numpy as np
import ml_dtypes

import concourse.mybir as mybir
import concourse.tile as tile
from concourse import bacc
from concourse.bass_utils import run_bass_kernel_spmd

N, F, D, P, E, CORES, SEM_H = 50000, 256, 256, 4, 800000, 8, 128
NC = N // CORES            # 6250 dst nodes per core
CAP = 125                  # nodes per block (3 pad slots each)
NB = NC // CAP             # 50 blocks
NCP = NB * 128             # 6400 padded dst slots per core
FH = F // 128              # 2 feature halves
DH = D // 128              # 2 output halves
GRP = 2                    # dst blocks per et-stream DMA

_nc_cache = {}
_last_in_maps = None


def _n_tiles():
    tiles = []
    off = 0
    while off < NCP:
        t = min(512, NCP - off)
        tiles.append((off, t))
        off += t
    return tiles


def _build(KBs):
    ntiles = _n_tiles()

    nc = bacc.Bacc("TRN2", target_bir_lowering=False, debug=False,
                   num_devices=CORES)
    dt = mybir.dt
    et_in, dst_in = [], []
    for p in range(P):
        CHP = NB * KBs[p]
        et_in.append(nc.dram_tensor(f"et{p}", [128, CHP * F], dt.bfloat16,
                                    kind="ExternalInput").ap())
        dst_in.append(nc.dram_tensor(f"dst{p}", [128, CHP], dt.bfloat16,
                                     kind="ExternalInput").ap())
    Wgc = nc.dram_tensor("Wgc", [P, F, D], dt.float32,
                         kind="ExternalInput").ap()
    W1 = nc.dram_tensor("W1", [D, SEM_H], dt.float32,
                        kind="ExternalInput").ap()
    b1p = nc.dram_tensor("b1p", [SEM_H, P], dt.float32,
                         kind="ExternalInput").ap()
    w2 = nc.dram_tensor("w2", [SEM_H, 1], dt.float32,
                        kind="ExternalInput").ap()
    iota_in = nc.dram_tensor("iota", [128, 128], dt.bfloat16,
                             kind="ExternalInput").ap()
    ident_in = nc.dram_tensor("ident", [128, 128], dt.float32,
                              kind="ExternalInput").ap()
    s4i_in = nc.dram_tensor("s4i", [1, 128], dt.float32,
                            kind="ExternalInput").ap()
    out = nc.dram_tensor("out", [D, NCP], dt.bfloat16,
                         kind="ExternalOutput").ap()

    with tile.TileContext(nc) as tc:
        with (
            tc.tile_pool(name="const", bufs=1) as cp,
            tc.tile_pool(name="stage", bufs=2) as stp,
            tc.tile_pool(name="meta2", bufs=2) as mp2,
            tc.tile_pool(name="edges", bufs=3) as ep,
            tc.tile_pool(name="sel", bufs=4) as selp,
            tc.tile_pool(name="work", bufs=4) as wp,
            tc.tile_pool(name="psum_acc", bufs=3, space="PSUM") as pa,
            tc.tile_pool(name="psum_tp", bufs=2, space="PSUM") as pt,
            tc.tile_pool(name="psum_mm", bufs=3, space="PSUM") as pm,
            tc.tile_pool(name="dram", bufs=2, space="DRAM") as dp,
        ):
            # ---- constants ----
            iota_b = cp.tile([128, 128], dt.bfloat16)
            nc.sync.dma_start(out=iota_b[:], in_=iota_in[:])
            ident = cp.tile([128, 128], dt.float32)
            nc.sync.dma_start(out=ident[:], in_=ident_in[:])
            ones1 = cp.tile([1, 128], dt.float32)
            nc.vector.memset(ones1[:], 1.0)

            wgc_sb = []
            for p in range(P):
                per_fh = []
                for fh in range(FH):
                    t32 = stp.tile([128, D], dt.float32)
                    nc.sync.dma_start(out=t32[:],
                                      in_=Wgc[p, fh * 128:(fh + 1) * 128, :])
                    t16 = cp.tile([128, D], dt.bfloat16, name=f"wgc{p}_{fh}")
                    nc.vector.tensor_copy(out=t16[:], in_=t32[:])
                    per_fh.append(t16)
                wgc_sb.append(per_fh)
            w1_sb = []
            for dh in range(DH):
                t32 = stp.tile([128, SEM_H], dt.float32)
                nc.sync.dma_start(out=t32[:],
                                  in_=W1[dh * 128:(dh + 1) * 128, :])
                t16 = cp.tile([128, SEM_H], dt.bfloat16, name=f"w1_{dh}")
                nc.vector.tensor_copy(out=t16[:], in_=t32[:])
                w1_sb.append(t16)
            b1p_sb = cp.tile([128, P], dt.float32)
            nc.sync.dma_start(out=b1p_sb[:], in_=b1p[:])
            w2_32 = stp.tile([128, 1], dt.float32)
            nc.sync.dma_start(out=w2_32[:], in_=w2[:])
            w2_sb = cp.tile([128, 1], dt.bfloat16)
            nc.vector.tensor_copy(out=w2_sb[:], in_=w2_32[:])

            zT_sb = [cp.tile([128, DH * NCP], dt.bfloat16, name=f"zT{p}")
                     for p in range(P)]
            # pad-slot semantic-score correction, precomputed on host
            s4_sb = cp.tile([1, 128], dt.float32)
            nc.sync.dma_start(out=s4_sb[:], in_=s4i_in[:])

            dst_sbs = []
            for p in range(P):
                dst_sb = mp2.tile([128, NB * KBs[p]], dt.bfloat16,
                                  tag=f"dst{p}")
                nc.sync.dma_start(out=dst_sb[:], in_=dst_in[p][:])
                dst_sbs.append(dst_sb)

            # ---- main: aggregation + per-metapath transform ----
            for p in range(P):
                KB = KBs[p]
                dst_sb = dst_sbs[p]

                zT3 = zT_sb[p][:].rearrange("q (d n) -> q d n", d=DH)

                def _t_stage(b, agg_tmp):
                    # both transposes into one PSUM tile, one ACT copy
                    tp_ps = pt.tile([128, FH * 128], dt.float32, tag="tpz")
                    for fh in range(FH):
                        nc.tensor.transpose(
                            out=tp_ps[:, fh * 128:(fh + 1) * 128],
                            in_=agg_tmp[:, fh * 128:(fh + 1) * 128],
                            identity=ident[:])
                    aggT_blk = wp.tile([128, FH * 128], dt.bfloat16,
                                       tag="aggT")
                    nc.scalar.activation(
                        out=aggT_blk[:], in_=tp_ps[:],
                        func=mybir.ActivationFunctionType.Copy)
                    return aggT_blk

                def _w_stage(b, aggT_blk):
                    # zT(block) = W^T @ aggT  (bias folded into b1p);
                    # both dh halves in one PSUM tile, one strided ACT copy
                    zp = pt.tile([128, DH * 128], dt.float32, tag="tpz")
                    for dh in range(DH):
                        for fh in range(FH):
                            nc.tensor.matmul(
                                out=zp[:, dh * 128:(dh + 1) * 128],
                                lhsT=wgc_sb[p][fh][:,
                                                   dh * 128:(dh + 1) * 128],
                                rhs=aggT_blk[:, fh * 128:(fh + 1) * 128],
                                start=(fh == 0), stop=(fh == FH - 1))
                    nc.scalar.activation(
                        out=zT3[:, :, b * 128:(b + 1) * 128],
                        in_=zp[:].rearrange("q (d n) -> q d n", d=DH),
                        func=mybir.ActivationFunctionType.Copy)

                pend1 = pend2 = None
                b0 = 0
                while b0 < NB:
                    ng = min(GRP, NB - b0)
                    et = ep.tile([128, GRP * KB * F], dt.bfloat16)
                    nc.sync.dma_start(
                        out=et[:, :ng * KB * F],
                        in_=et_in[p][:, b0 * KB * F:(b0 + ng) * KB * F])
                    for i in range(ng):
                        b = b0 + i
                        sel = selp.tile([128, KB * 128], dt.bfloat16)
                        nc.vector.tensor_tensor(
                            out=sel[:].rearrange("q (k j) -> q k j", k=KB),
                            in0=iota_b[:].unsqueeze(1).broadcast_to(
                                (128, KB, 128)),
                            in1=dst_sb[:, b * KB:(b + 1) * KB].unsqueeze(
                                2).broadcast_to((128, KB, 128)),
                            op=mybir.AluOpType.is_equal)
                        acc = pa.tile([128, F], dt.float32, name="acc")
                        for k in range(KB):
                            nc.tensor.matmul(
                                out=acc[:],
                                lhsT=sel[:, k * 128:(k + 1) * 128],
                                rhs=et[:, (i * KB + k) * F:
                                       (i * KB + k + 1) * F],
                                start=(k == 0), stop=(k == KB - 1))
                        agg_tmp = wp.tile([128, F], dt.float32, tag="aggtmp")
                        nc.scalar.activation(
                            out=agg_tmp[:], in_=acc[:],
                            func=mybir.ActivationFunctionType.Copy)
                        if pend2 is not None:
                            _w_stage(*pend2)
                            pend2 = None
                        if pend1 is not None:
                            pend2 = (pend1[0], _t_stage(*pend1))
                            pend1 = None
                        pend1 = (b, agg_tmp)
                    b0 += ng
                if pend2 is not None:
                    _w_stage(*pend2)
                if pend1 is not None:
                    _w_stage(pend1[0], _t_stage(*pend1))

                # semantic attention scores: s = tanh(z@W1 + b1') @ w2
                for (n0, nt) in ntiles:
                    tp = pm.tile([128, 512], dt.float32, tag="mm")
                    for dh in range(DH):
                        nc.tensor.matmul(
                            out=tp[:, :nt],
                            lhsT=w1_sb[dh][:],
                            rhs=zT_sb[p][:, dh * NCP + n0:dh * NCP + n0 + nt],
                            start=(dh == 0), stop=(dh == DH - 1))
                    t_sb = wp.tile([128, 512], dt.bfloat16)
                    nc.scalar.activation(
                        out=t_sb[:, :nt], in_=tp[:, :nt],
                        func=mybir.ActivationFunctionType.Tanh,
                        bias=b1p_sb[:, p:p + 1])
                    sp = pm.tile([1, 512], dt.float32, tag="mm")
                    nc.tensor.matmul(out=sp[:, :nt], lhsT=w2_sb[:],
                                     rhs=t_sb[:, :nt], start=True, stop=True)
                    stmp = wp.tile([1, 1], dt.float32)
                    nc.vector.tensor_reduce(
                        out=stmp[:], in_=sp[:, :nt],
                        axis=mybir.AxisListType.X, op=mybir.AluOpType.add)
                    nc.vector.tensor_tensor(
                        out=s4_sb[:, p:p + 1], in0=s4_sb[:, p:p + 1],
                        in1=stmp[:], op=mybir.AluOpType.add)

            # ---- semantic softmax over metapaths (global mean via AllReduce)
            cc_in = dp.tile([1, 128], dt.float32)
            cc_out = dp.tile([1, 128], dt.float32)
            nc.sync.dma_start(out=cc_in[:], in_=s4_sb[:])
            nc.gpsimd.collective_compute(
                "AllReduce", mybir.AluOpType.add,
                replica_groups=[list(range(CORES))],
                ins=[cc_in.opt()], outs=[cc_out.opt()])
            sall = wp.tile([1, 128], dt.float32)
            nc.sync.dma_start(out=sall[:], in_=cc_out[:])

            bexp = wp.tile([1, P], dt.float32)
            nc.scalar.activation(out=bexp[:], in_=sall[:, :P],
                                 func=mybir.ActivationFunctionType.Exp,
                                 scale=1.0 / N)
            bsum = wp.tile([1, 1], dt.float32)
            nc.vector.tensor_reduce(out=bsum[:], in_=bexp[:],
                                    axis=mybir.AxisListType.X,
                                    op=mybir.AluOpType.add)
            binv = wp.tile([1, 1], dt.float32)
            nc.vector.reciprocal(out=binv[:], in_=bsum[:])
            bnorm = wp.tile([1, P], dt.float32)
            nc.vector.tensor_scalar_mul(out=bnorm[:], in0=bexp[:],
                                        scalar1=binv[:, 0:1])
            bb_ps = pm.tile([128, P], dt.float32, tag="mm")
            nc.tensor.matmul(out=bb_ps[:], lhsT=ones1[:], rhs=bnorm[:],
                             start=True, stop=True)
            bb_sb = wp.tile([128, P], dt.float32)
            nc.vector.tensor_copy(out=bb_sb[:], in_=bb_ps[:])
            diag = []
            for p in range(P):
                dg = cp.tile([128, 128], dt.bfloat16, name=f"diag{p}")
                nc.vector.tensor_scalar_mul(out=dg[:], in0=ident[:],
                                            scalar1=bb_sb[:, p:p + 1])
                diag.append(dg)

            # ---- weighted combine + output ----
            for dh in range(DH):
                for (n0, nt) in ntiles:
                    op_ps = pm.tile([128, 512], dt.float32, tag="mm")
                    for p in range(P):
                        nc.tensor.matmul(
                            out=op_ps[:, :nt], lhsT=diag[p][:],
                            rhs=zT_sb[p][:, dh * NCP + n0:dh * NCP + n0 + nt],
                            start=(p == 0), stop=(p == P - 1))
                    ot = wp.tile([128, 512], dt.bfloat16)
                    nc.vector.tensor_copy(out=ot[:, :nt], in_=op_ps[:, :nt])
                    nc.sync.dma_start(
                        out=out[dh * 128:(dh + 1) * 128, n0:n0 + nt],
                        in_=ot[:, :nt])
    nc.compile()
    return nc


def _balance(deg, caps):
    """Assign NC nodes to NB blocks, balancing all P per-metapath in-degree
    sums simultaneously (greedy, heaviest node first)."""
    order = np.argsort(-deg.sum(axis=0), kind="stable")
    loads = np.zeros((NB, deg.shape[0]), dtype=np.int64)
    counts = np.zeros(NB, dtype=np.int64)
    assign = np.empty(NC, dtype=np.int64)
    for n in order:
        feas = counts < caps
        newmax = np.where(feas[:, None], loads + deg[:, n],
                          1 << 40).max(axis=1)
        b = int(np.argmin(newmax))
        assign[n] = b
        loads[b] += deg[:, n]
        counts[b] += 1
    return assign, loads.max(axis=0)


def _prep_core(h32, src_p, dst_p, w_p, base, KB, blk_of, pos_of):
    """Per-core, per-metapath host staging: returns (et [128, CHP*F] bf16,
    dstpos [128, CHP] bf16) in block-major chunk layout."""
    CHP = NB * KB
    m = (dst_p >= base) & (dst_p < base + NC)
    s, d, w = src_p[m], dst_p[m] - base, w_p[m]
    blk = blk_of[d]
    order = np.argsort(blk, kind="stable")
    s, d, w, blk = s[order], d[order], w[order], blk[order]
    cnt = np.bincount(blk, minlength=NB)
    start = np.concatenate([[0], np.cumsum(cnt)])[:-1]
    r = np.arange(len(d)) - start[blk]          # rank within block
    assert cnt.max() <= KB * 128
    slot = (blk * KB + r // 128) * 128 + r % 128

    dstpos = np.full(CHP * 128, -1.0, dtype=np.float32)
    dstpos[slot] = pos_of[d]
    et = np.zeros((CHP * 128, F), dtype=ml_dtypes.bfloat16)
    et[slot] = (h32[s] * w[:, None]).astype(ml_dtypes.bfloat16)
    et = np.ascontiguousarray(
        et.reshape(CHP, 128, F).transpose(1, 0, 2)).reshape(128, CHP * F)
    dstpos = np.ascontiguousarray(
        dstpos.reshape(CHP, 128).T).astype(ml_dtypes.bfloat16)
    return et, dstpos


def kernel(h, src, dst, W_gc, b_gc, W1, b1, w2):
    h = np.ascontiguousarray(h, dtype=np.float32)
    src = np.asarray(src)
    dst = np.asarray(dst)
    W_gc = np.ascontiguousarray(W_gc, dtype=np.float32)
    b_gc = np.asarray(b_gc, dtype=np.float32)
    W1 = np.ascontiguousarray(W1, dtype=np.float32)
    b1 = np.asarray(b1, dtype=np.float32)
    w2 = np.asarray(w2, dtype=np.float32)

    w_edge = []
    for p in range(P):
        deg_out = np.clip(np.bincount(src[p], minlength=N), 1, None)
        deg_in = np.clip(np.bincount(dst[p], minlength=N), 1, None)
        w_edge.append((1.0 / np.sqrt(deg_out[src[p]]) /
                       np.sqrt(deg_in[dst[p]])).astype(np.float32))

    caps = np.full(NB, CAP, dtype=np.int64)
    blk_of, pos_of = [], []
    maxload = np.zeros(P, dtype=np.int64)
    for c in range(CORES):
        base = c * NC
        degs = []
        for p in range(P):
            m = (dst[p] >= base) & (dst[p] < base + NC)
            degs.append(np.bincount(dst[p][m] - base, minlength=NC))
        assign, mx = _balance(np.stack(degs), caps)
        maxload = np.maximum(maxload, mx)
        order = np.argsort(assign, kind="stable")
        pos = np.empty(NC, dtype=np.int64)
        starts = np.concatenate([[0], np.cumsum(np.bincount(assign,
                                                            minlength=NB))])
        pos[order] = np.arange(NC) - starts[assign[order]]
        blk_of.append(assign)
        pos_of.append(pos.astype(np.float32))
    KBs = tuple(max(1, int(-(-maxload[p] // 128))) for p in range(P))

    if KBs not in _nc_cache:
        _nc_cache[KBs] = _build(KBs)
    nc = _nc_cache[KBs]

    b1p = np.stack([b1 + W1.T @ b_gc[p] for p in range(P)], axis=1)
    iota = np.arange(128, dtype=np.float32)[None, :].repeat(128, axis=0)
    ident = np.eye(128, dtype=np.float32)
    # pad slots contribute tanh(b1p)@w2 each to the per-core score sum
    s4i = np.zeros((1, 128), dtype=np.float32)
    s4i[0, :P] = -(NCP - NC) * (np.tanh(b1p.T.astype(np.float32)) @ w2)

    in_maps = []
    for c in range(CORES):
        base = c * NC
        im = {
            "Wgc": W_gc,
            "W1": W1,
            "b1p": b1p,
            "w2": w2.reshape(SEM_H, 1),
            "iota": iota.astype(ml_dtypes.bfloat16),
            "ident": ident,
            "s4i": s4i,
        }
        for p in range(P):
            et, dstpos = _prep_core(h, src[p], dst[p], w_edge[p], base,
                                    KBs[p], blk_of[c], pos_of[c])
            im[f"et{p}"] = et
            im[f"dst{p}"] = dstpos
        in_maps.append(im)

    global _last_in_maps
    _last_in_maps = in_maps
    res = run_bass_kernel_spmd(nc, in_maps, list(range(CORES))).results
    out = np.empty((N, D), dtype=np.float32)
    for c in range(CORES):
        slot = blk_of[c] * 128 + pos_of[c].astype(np.int64)
        out[c * NC:(c + 1) * NC] = res[c]["out"][:, slot].T.astype(np.float32)
    return out
